# revision 38
# baseline (speedup 1.0000x reference)
"""Per-subject linear dispatch (MoE-style routing) + masked token blend.

Computes, for B=32 samples sharded 4-per-core across 8 NeuronCores:
    h   = x @ W[subject_ids] + b[subject_ids]          # [B, S, D]
    h   = h * (1 - mask) + mask_token * mask
    out = concat([subj_table[subject_ids][:, None, :], h], axis=1)

Strategy: the whole elementwise epilogue is folded into the GEMM by
augmenting the contraction dim with 2 rows:
    x_aug = [x * (1-m), (1-m), m]           # [S, C+2]
    W_aug = [W; b; mask_token]              # [C+2, D]
    h_final = x_aug @ W_aug  (exactly)
The host does the (free) gather/transpose/scale; the device runs a pure
batched GEMM with K=514 = 4x128 + 2, M=128-row S-tiles, N=512 D-tiles,
accumulated in PSUM. The subject-embedding row is a host-side gather.
"""

import os
from contextlib import ExitStack

import numpy as np

import concourse.bass as bass
import concourse.mybir as mybir
import concourse.tile as tile
from concourse import bacc
from concourse.bass_utils import run_bass_kernel_spmd

B, S, C, D = 32, 512, 512, 1024
NCORES = 8
BPC = B // NCORES          # samples per core
KAUG = C + 2               # augmented contraction dim (unpacked: 1-m, m rows)
P = 128
NKC = C // P               # full K chunks of 128
FD = 512                   # matmul moving free dim (one PSUM bank)
ND = D // FD
NST = S // P

# Packed path: masked rows (mask==1) produce exactly mask_token, so only
# unmasked rows go through the GEMM. U = padded row budget (3 tiles of 128;
# P(Binomial(512,.5) > 384) ~ 1e-31, with an unpacked fallback regardless).
U = 384
NST_P = U // P
KAUG_P = C + 1             # just the all-ones bias row

# matmul input dtype: "float32" (exact, 4 cyc/row), "float16"/"bfloat16"
# (1 cyc/row, host-side cast, halved input DMA), or "float32r" (1 cyc/row at
# N>=256, fp32 storage + on-device rounding pass).
MM_DTYPE = os.environ.get("BASS_MM_DTYPE", "float16")

_NP_DT = {
    "float32": np.float32,
    "float32r": np.float32,
    "float16": np.float16,
    "bfloat16": None,  # ml_dtypes.bfloat16, resolved lazily
}


def _np_in_dtype(name):
    if name == "bfloat16":
        import ml_dtypes

        return ml_dtypes.bfloat16
    return _NP_DT[name]

TRACE = False
LAST_EXEC_NS = None
LAST_RESULTS = None

_nc_cache = {}


def _build(mm_dtype_name: str, packed: bool):
    mm_dt = getattr(mybir.dt, mm_dtype_name)
    # storage dtype of the DRAM inputs / SBUF tiles
    in_dt = mybir.dt.float32 if mm_dtype_name in ("float32", "float32r") else mm_dt
    round_pass = mm_dtype_name == "float32r"

    s_dim = U if packed else S            # per-sample GEMM row count
    kaug = KAUG_P if packed else KAUG
    naug = kaug - C                       # 1 (packed) or 2 (unpacked)
    nst = s_dim // P

    nc = bacc.Bacc(
        "TRN2",
        target_bir_lowering=False,
        debug=False,
        num_devices=NCORES,
    )
    # Host pre-chunks so each SBUF partition's data is one contiguous DRAM
    # run: xT[b, p, kc, s] = x_aug[s, kc*128+p].
    xT = nc.dram_tensor("xT", [BPC, P, NKC, s_dim], in_dt, kind="ExternalInput").ap()
    w = nc.dram_tensor("w", [BPC, P, NKC, D], in_dt, kind="ExternalInput").ap()
    xa_d = nc.dram_tensor("xa", [BPC, naug, s_dim], in_dt, kind="ExternalInput").ap()
    wa_d = nc.dram_tensor("wa", [BPC, naug, D], in_dt, kind="ExternalInput").ap()
    out = nc.dram_tensor(
        "out", [BPC, s_dim, D], mybir.dt.float32, kind="ExternalOutput"
    ).ap()

    with ExitStack() as ctx:
        tc = ctx.enter_context(tile.TileContext(nc))
        xp = ctx.enter_context(tc.tile_pool(name="xp", bufs=3))
        wp = ctx.enter_context(tc.tile_pool(name="wp", bufs=3))
        ap_ = ctx.enter_context(tc.tile_pool(name="augp", bufs=3))
        pp = ctx.enter_context(tc.tile_pool(name="pp", bufs=8, space="PSUM"))
        op = ctx.enter_context(tc.tile_pool(name="op", bufs=3))

        for bb in range(BPC):
            # Whole-sample SBUF residency; single large DMA per tensor.
            # Inputs ride the SP HWDGE ring; outputs ride the ACT ring so
            # compute-gated stores never block the next sample's prefetch
            # (HWDGE rings are FIFO per issuing engine).
            xt = xp.tile([P, NKC, s_dim], in_dt, name="xt")
            wt = wp.tile([P, NKC, D], in_dt, name="wt")
            xa = ap_.tile([naug, s_dim], in_dt, name="xa")
            wa = ap_.tile([naug, D], in_dt, name="wa")
            nc.sync.dma_start(xt[:], xT[bb])
            nc.sync.dma_start(wt[:], w[bb])
            nc.sync.dma_start(xa[:], xa_d[bb])
            nc.sync.dma_start(wa[:], wa_d[bb])

            if round_pass:
                # fp32r inputs must be produced by an instruction that
                # rounds to fp32r; DVE copy with fp32r output dtype.
                xtr = xp.tile([P, NKC, s_dim], mybir.dt.float32r, name="xtr")
                wtr = wp.tile([P, NKC, D], mybir.dt.float32r, name="wtr")
                xar = ap_.tile([naug, s_dim], mybir.dt.float32r, name="xar")
                war = ap_.tile([naug, D], mybir.dt.float32r, name="war")
                nc.vector.tensor_copy(xtr[:], xt[:])
                nc.vector.tensor_copy(wtr[:], wt[:])
                nc.vector.tensor_copy(xar[:], xa[:])
                nc.vector.tensor_copy(war[:], wa[:])
                xt, wt, xa, wa = xtr, wtr, xar, war

            for st in range(nst):
                ot = op.tile([P, D], mybir.dt.float32, name="ot")
                for dd in range(ND):
                    ps = pp.tile([P, FD], mybir.dt.float32, name="ps")
                    for kc in range(NKC):
                        nc.tensor.matmul(
                            ps[:],
                            xt[:, kc, st * P:(st + 1) * P],
                            wt[:, kc, dd * FD:(dd + 1) * FD],
                            start=(kc == 0),
                            stop=False,
                        )
                    nc.tensor.matmul(
                        ps[:],
                        xa[:, st * P:(st + 1) * P],
                        wa[:, dd * FD:(dd + 1) * FD],
                        start=False,
                        stop=True,
                    )
                    # copyback split across ACT and DVE so neither binds
                    if dd == 0:
                        nc.scalar.copy(ot[:, dd * FD:(dd + 1) * FD], ps[:])
                    else:
                        nc.vector.tensor_copy(ot[:, dd * FD:(dd + 1) * FD], ps[:])
                nc.scalar.dma_start(out[bb, st * P:(st + 1) * P, :], ot[:])
    nc.compile()
    return nc


def _build_raw(mm_dtype_name: str, packed: bool):
    """Hand-scheduled variant (no TileContext): avoids the Tile kernel-tail
    drain + EVSEM butterfly (~10us) and the start barrier, and streams the
    first sample's K-chunks so the PE starts as early as possible.

    Engine plan per core:
      SP   - all input DMAs (HWDGE ring, FIFO)
      PE   - 5 matmuls per PSUM group (4 K-chunks + 1 aug row chunk)
      ACT  - copyback of dd=0 halves + all output DMAs (own HWDGE ring)
      DVE  - copyback of dd=1 halves
    All xt/wt/ot buffers are distinct SBUF tensors (everything fits), so the
    only reuse hazard is the 8 PSUM banks (24 groups), handled with
    copy-completion semaphores.
    """
    mm_dt = getattr(mybir.dt, mm_dtype_name)
    assert mm_dtype_name not in ("float32r",), "raw impl: no fp32r round pass"
    in_dt = mm_dt if mm_dtype_name != "float32" else mybir.dt.float32

    s_dim = U if packed else S
    kaug = KAUG_P if packed else KAUG
    naug = kaug - C
    nst = s_dim // P
    ngrp = BPC * nst                      # (sample, st) pairs; x2 dd = psum groups

    nc = bacc.Bacc(
        "TRN2",
        target_bir_lowering=False,
        debug=False,
        num_devices=NCORES,
    )
    xT = nc.dram_tensor("xT", [BPC, P, NKC, s_dim], in_dt, kind="ExternalInput").ap()
    w = nc.dram_tensor("w", [BPC, P, NKC, D], in_dt, kind="ExternalInput").ap()
    xa_d = nc.dram_tensor("xa", [BPC, naug, s_dim], in_dt, kind="ExternalInput").ap()
    wa_d = nc.dram_tensor("wa", [BPC, naug, D], in_dt, kind="ExternalInput").ap()
    out = nc.dram_tensor(
        "out", [BPC, s_dim, D], mybir.dt.float32, kind="ExternalOutput"
    ).ap()

    # SBUF/PSUM allocations (flat, whole-kernel lifetime)
    # Spreading aug rows across PE row groups {0,32,64} measured SLOWER on HW
    # (59.6us vs 52.9us): sample-end aug grouping delays the copyback stream
    # and stalls PSUM-bank reuse. Keep the inline per-group aug matmul.
    aug_spread = False
    aug_parts = 32 * (nst - 1) + naug if aug_spread else naug
    xt = [nc.alloc_sbuf_tensor(f"xt{b}", [P, NKC, s_dim], in_dt).ap() for b in range(BPC)]
    wt = [nc.alloc_sbuf_tensor(f"wt{b}", [P, NKC, D], in_dt).ap() for b in range(BPC)]
    xa = [nc.alloc_sbuf_tensor(f"xa{b}", [aug_parts, s_dim], in_dt).ap() for b in range(BPC)]
    wa = [nc.alloc_sbuf_tensor(f"wa{b}", [aug_parts, D], in_dt).ap() for b in range(BPC)]
    ot = [nc.alloc_sbuf_tensor(f"ot{n}", [P, D], mybir.dt.float32).ap() for n in range(ngrp)]
    bias_sb = [
        nc.alloc_sbuf_tensor(f"bias{b}", [P, D], mybir.dt.float32).ap()
        for b in range(BPC)
    ]
    scratch = nc.alloc_sbuf_tensor("scratch", [P, FD], in_dt).ap()
    ps = [nc.alloc_psum_tensor(f"ps{k}", [P, FD], mybir.dt.float32).ap() for k in range(8)]

    # HWDGE DMA +16 increments are not atomic across concurrent DMAs, so a
    # shared counting semaphore with intermediate thresholds is racy. Use one
    # semaphore per wait-group, always waited at its full total.
    # sample 0 is chunk-streamed: sem per (xt,wt) chunk pair; samples 1..:
    # one sem for the whole sample (aug + xt + wt).
    aug0_sem = nc.alloc_semaphore("aug0_sem")                      # total 32
    pair_sems = [nc.alloc_semaphore(f"p0k{k}") for k in range(NKC)]  # 32 each
    samp_sems = [nc.alloc_semaphore(f"samp{b}") for b in range(1, BPC)]  # 64
    mm_done = nc.alloc_semaphore("mm_done")
    bias_mm = nc.alloc_semaphore("bias_mm")
    bias_cp = nc.alloc_semaphore("bias_cp")
    copy_dve = nc.alloc_semaphore("copy_dve")
    out_sem = nc.alloc_semaphore("out_sem")
    scratch_sem = nc.alloc_semaphore("scratch_sem")

    # PSUM plan: GEMM groups cycle banks 0-3 ((2n+dd)%4); per-sample bias
    # broadcasts (ones.T @ b_row via K=1 matmuls) live in banks 4-7,
    # double-buffered by sample parity. The bias add is folded into the DVE
    # copyback (out_sbuf = group_psum + bias_psum), so the per-group K=1 aug
    # matmuls disappear: 2 bias MMs per sample instead of 2 per group.
    def gbank(n, dd):
        return (2 * n + dd) % 4

    def bbank(b, dd):
        return 4 + (b % 2) * 2 + dd

    with nc.Block() as block:

        def aug_dma(sp, b, sem):
            cnt = 0
            if aug_spread:
                for g in range(nst):
                    sp.dma_start(
                        xa[b][32 * g:32 * g + naug, :], xa_d[b]
                    ).then_inc(sem, 16)
                    sp.dma_start(
                        wa[b][32 * g:32 * g + naug, :], wa_d[b]
                    ).then_inc(sem, 16)
                    cnt += 32
            else:
                sp.dma_start(xa[b][:], xa_d[b]).then_inc(sem, 16)
                sp.dma_start(wa[b][:], wa_d[b]).then_inc(sem, 16)
                cnt = 32
            return cnt

        samp_total = {}

        @block.sync
        def _(sp):
            # sample 0 chunk-streamed, first (xt,wt) K-chunk pair first so
            # the PE can start after ~0.4MB; aug rows are only needed at the
            # end of the first accumulation group.
            for kc in range(NKC):
                sp.dma_start(xt[0][:, kc, :], xT[0, :, kc, :]).then_inc(
                    pair_sems[kc], 16
                )
                sp.dma_start(wt[0][:, kc, :], w[0, :, kc, :]).then_inc(
                    pair_sems[kc], 16
                )
            samp_total[0] = aug_dma(sp, 0, aug0_sem)
            for b in range(1, BPC):
                sem = samp_sems[b - 1]
                cnt = aug_dma(sp, b, sem)
                sp.dma_start(xt[b][:], xT[b]).then_inc(sem, 16)
                sp.dma_start(wt[b][:], w[b]).then_inc(sem, 16)
                samp_total[b] = cnt + 32
            # output DMAs: even groups ride the tail of the SP ring (inputs
            # are already enqueued ahead), odd groups go out on the ACT ring
            # (below) so the two rings transfer and drain in parallel.
            # No explicit wait on out_sem: the DMA-completion semaphore lands
            # ~6us after the data (HBM WAW-visibility path), while the
            # framework's end-of-program DRAIN on each issuing engine already
            # empties its HWDGE ring before the NEFF completes.
            for n in range(0, ngrp, 2):
                sp.wait_ge(copy_dve, 2 * n + 2)
                b, st = divmod(n, nst)
                sp.dma_start(out[b, st * P:(st + 1) * P, :], ot[n][:]).then_inc(
                    out_sem, 16
                )

        @block.gpsimd
        def _(gps):
            gps.memset(scratch[:], 0.0).then_inc(scratch_sem, 1)

        @block.tensor
        def _(pe):
            seen = set()

            def need(sem, val):
                if (sem, val) not in seen:
                    pe.wait_ge(sem, val)
                    seen.add((sem, val))

            # HAM warm-up: spin zero-matmuls into the (still free) sample-0
            # bias bank while the first input DMAs are in flight, so the PE
            # clock-gate is at 8/8 by the time real data lands (~5us fill).
            pe.wait_ge(scratch_sem, 1)
            for _ in range(12):
                pe.matmul(
                    ps[bbank(0, 0)][:],
                    scratch[:, 0:P],
                    scratch[:],
                    start=True,
                    stop=True,
                )

            def bias_mms(b):
                # wa[b] arrival + bias-bank reuse (sample b-2's adds done)
                if b == 0:
                    need(aug0_sem, samp_total[0])
                else:
                    need(samp_sems[b - 1], samp_total[b])
                if b >= 2:
                    pe.wait_ge(bias_cp, b - 1)
                for dd in range(ND):
                    mm = pe.matmul(
                        ps[bbank(b, dd)][:],
                        xa[0][:, 0:P],
                        wa[b][:, dd * FD:(dd + 1) * FD],
                        start=True,
                        stop=True,
                    )
                    if dd == ND - 1:
                        mm.then_inc(bias_mm, 1)

            for b in range(BPC):
                # sample 0: don't delay the first GEMM group on the aug DMAs
                # (they are queued after the K-chunk pairs); emit its bias
                # matmuls after the first group instead.
                if b > 0:
                    bias_mms(b)
                for st in range(nst):
                    n = b * nst + st
                    # PSUM bank reuse: wait for the adds of the group pair
                    # 2 n-steps earlier to finish.
                    if n >= 2:
                        pe.wait_ge(copy_dve, 2 * (n - 2) + 2)
                    # dd pairs share the stationary operand per K-chunk,
                    # interleaving both banks' accumulation groups.
                    for kc in range(NKC):
                        if b == 0:
                            need(pair_sems[kc], 32)
                        else:
                            need(samp_sems[b - 1], samp_total[b])
                        for dd in range(ND):
                            mm = pe.matmul(
                                ps[gbank(n, dd)][:],
                                xt[b][:, kc, st * P:(st + 1) * P],
                                wt[b][:, kc, dd * FD:(dd + 1) * FD],
                                start=(kc == 0),
                                stop=(kc == NKC - 1),
                            )
                            if kc == NKC - 1:
                                mm.then_inc(mm_done, 1)
                    if b == 0 and st == 0:
                        bias_mms(0)

        @block.scalar
        def _(act):
            # ACT stages each sample's bias broadcast PSUM -> SBUF (HW allows
            # only one PSUM operand per compute instruction, so the DVE add
            # needs the bias in SBUF).
            for b in range(BPC):
                act.wait_ge(bias_mm, b + 1)
                for dd in range(ND):
                    cp = act.copy(
                        bias_sb[b][:, dd * FD:(dd + 1) * FD], ps[bbank(b, dd)][:]
                    )
                    if dd == ND - 1:
                        cp.then_inc(bias_cp, 1)
            # odd output groups on the ACT HWDGE ring (all bias copies are
            # done long before the first wait here can block the FIFO)
            for n in range(1, ngrp, 2):
                act.wait_ge(copy_dve, 2 * n + 2)
                b, st = divmod(n, nst)
                act.dma_start(out[b, st * P:(st + 1) * P, :], ot[n][:]).then_inc(
                    out_sem, 16
                )

        @block.vector
        def _(dve):
            biased = set()
            for n in range(ngrp):
                b = n // nst
                if b not in biased:
                    dve.wait_ge(bias_cp, b + 1)
                    biased.add(b)
                for dd in range(ND):
                    dve.wait_ge(mm_done, 2 * n + dd + 1)
                    dve.tensor_add(
                        ot[n][:, dd * FD:(dd + 1) * FD],
                        ps[gbank(n, dd)][:],
                        bias_sb[b][:, dd * FD:(dd + 1) * FD],
                    ).then_inc(copy_dve, 1)

    nc.compile()
    return nc


IMPL = os.environ.get("BASS_IMPL", "raw")


def get_nc(mm_dtype_name: str | None = None, packed: bool = True):
    name = mm_dtype_name or MM_DTYPE
    key = (name, packed, IMPL)
    if key not in _nc_cache:
        # the unpacked fallback (mask distribution far from 50%) uses the
        # Tile builder, which is the variant validated on hardware for it
        builder = _build_raw if (IMPL == "raw" and packed) else _build
        _nc_cache[key] = builder(name, packed)
    return _nc_cache[key]


def _chunk_xT(xT_cs):
    """[B, C, s] (contraction-major) -> [B, P, NKC, s] per-partition-contiguous."""
    Bn, _, s_dim = xT_cs.shape
    return np.ascontiguousarray(
        xT_cs.reshape(Bn, NKC, P, s_dim).transpose(0, 2, 1, 3)
    )


def _chunk_w(w_cd):
    """[B, C, D] -> [B, P, NKC, D] per-partition-contiguous."""
    Bn = w_cd.shape[0]
    return np.ascontiguousarray(
        w_cd.reshape(Bn, NKC, P, D).transpose(0, 2, 1, 3)
    )


def _prepare_host_unpacked(x, one_m, m, W, b, mask_token, sid):
    np_dt = _np_in_dtype(MM_DTYPE)

    # x^T scaled by (1-m) along s: (C, S) per sample
    xT = _chunk_xT((x.transpose(0, 2, 1) * one_m[:, None, :]).astype(np_dt))
    xa = np.empty((B, 2, S), dtype=np_dt)
    xa[:, 0, :] = one_m.astype(np_dt)
    xa[:, 1, :] = m.astype(np_dt)

    w = _chunk_w(W[sid].astype(np_dt))
    wa = np.empty((B, 2, D), dtype=np_dt)
    wa[:, 0, :] = b[sid].astype(np_dt)
    wa[:, 1, :] = mask_token[0].astype(np_dt)
    return xT, w, xa, wa


def _prepare_host_packed(x, one_m, W, b, sid):
    """Keep only the first U rows per sample, unmasked ones first (stable
    argsort of the 0/1 mask). Trailing take-slots are real masked rows whose
    GEMM output is computed and discarded."""
    np_dt = _np_in_dtype(MM_DTYPE)

    take = np.argsort(one_m < 0.5, axis=1, kind="stable")[:, :U]   # [B, U]
    u = (one_m > 0.5).sum(axis=1).astype(np.int64)                 # [B]

    xg = x[np.arange(B)[:, None], take]                            # [B, U, C]
    xT = _chunk_xT(xg.transpose(0, 2, 1).astype(np_dt))
    xa = np.ones((B, 1, U), dtype=np_dt)

    w = _chunk_w(W[sid].astype(np_dt))
    wa = np.ascontiguousarray(b[sid].astype(np_dt)[:, None, :])
    return xT, w, xa, wa, take, u


def _run(nc, xT, w, xa, wa):
    global LAST_EXEC_NS, LAST_RESULTS
    in_maps = [
        {
            "xT": xT[c * BPC:(c + 1) * BPC],
            "w": w[c * BPC:(c + 1) * BPC],
            "xa": xa[c * BPC:(c + 1) * BPC],
            "wa": wa[c * BPC:(c + 1) * BPC],
        }
        for c in range(NCORES)
    ]
    res = run_bass_kernel_spmd(nc, in_maps, list(range(NCORES)), trace=TRACE)
    LAST_EXEC_NS = res.exec_time_ns
    LAST_RESULTS = res
    return np.concatenate([res.results[c]["out"] for c in range(NCORES)], axis=0)


def kernel(x, mask, W, b, subj_table, mask_token, subject_ids):
    x = np.asarray(x, dtype=np.float32)
    mask = np.asarray(mask, dtype=np.float32)
    W = np.asarray(W, dtype=np.float32)
    b = np.asarray(b, dtype=np.float32)
    subj_table = np.asarray(subj_table, dtype=np.float32)
    mask_token = np.asarray(mask_token, dtype=np.float32)
    sid = np.asarray(subject_ids).astype(np.int64)

    m = mask[:, :, 0]
    one_m = np.float32(1.0) - m

    out = np.empty((B, S + 1, D), dtype=np.float32)
    out[:, 0, :] = subj_table[sid]

    n_unmasked = int((one_m > 0.5).sum(axis=1).max())
    if n_unmasked <= U:
        xT, w, xa, wa, take, u = _prepare_host_packed(x, one_m, W, b, sid)
        dev = _run(get_nc(packed=True), xT, w, xa, wa)    # [B, U, D]
        # masked rows are exactly mask_token
        out[:, 1:, :] = mask_token[0]
        valid = np.arange(U)[None, :] < u[:, None]
        bidx, pos = np.nonzero(valid)
        out[bidx, 1 + take[bidx, pos], :] = dev[bidx, pos, :]
    else:
        xT, w, xa, wa = _prepare_host_unpacked(x, one_m, m, W, b, mask_token, sid)
        dev = _run(get_nc(packed=False), xT, w, xa, wa)   # [B, S, D]
        out[:, 1:, :] = dev
    return out
